# revision 2
# baseline (speedup 1.0000x reference)
"""BiGRU encoder (nn_BiGRUEncoder) as an 8-core TRN2 Bass kernel.

Contract: kernel(**inputs) takes the FULL unsharded inputs from
setup_inputs() and returns the FULL [B, T-2L, 2F] output, distributing work
across 8 NeuronCores internally.

Decomposition: the hidden dim F=1024 is split across the 8 cores (128
features each). Every core runs BOTH scan directions with the full batch
B=32, computing its 384 rows of the 3F gate pre-activations per step. After
each step the transposed h chunks ([128, 32] per direction) are exchanged
with an AllGather so the next step's recurrent matmul has the full h.T.
Input projections gi = x @ Wih.T don't depend on h and are hoisted into a
prologue as one large batched matmul per direction, stored in DRAM, and
streamed per step.

Per-step layouts: batch on partitions for gate math, with both directions
stacked ([64, X]: fwd rows 0-31, bwd rows 32-63); features on partitions for
the exchanged h.T chunks. The scan stops at T-L: the last L steps of either
direction feed no output.
"""

import sys

sys.path.insert(0, "/opt/trn_rl_repo")

import numpy as np

from concourse import bacc, tile, mybir
from concourse import bass_utils

F32 = mybir.dt.float32

B = 32  # batch
T = 512  # sequence length
F = 1024  # hidden/feature dim
L = 10  # trim at both ends of T
NC = 8  # cores
P = 128  # partitions / features per core
G = 3 * P  # gate rows per core
KB = F // P  # contraction blocks


def build_gru_kernel(nc, tc, with_gbias: bool, with_nbias: bool):
    """Emit the SPMD program (identical on all 8 cores)."""
    TS = T - L  # scan steps actually needed
    TO = T - 2 * L  # output steps

    xt = nc.dram_tensor("xt", [F, T * B], F32, kind="ExternalInput").ap()
    xo = nc.dram_tensor("xo", [T, B, P], F32, kind="ExternalInput").ap()
    wih = nc.dram_tensor("wih", [2, KB, P, G], F32, kind="ExternalInput").ap()
    whh = nc.dram_tensor("whh", [2, KB, P, G], F32, kind="ExternalInput").ap()
    ident = nc.dram_tensor("ident", [2 * B, 2 * B], F32, kind="ExternalInput").ap()
    if with_gbias:
        gbias = nc.dram_tensor("gbias", [2, P, G], F32, kind="ExternalInput").ap()
    if with_nbias:
        nbias = nc.dram_tensor("nbias", [2 * B, P], F32, kind="ExternalInput").ap()
    outp = nc.dram_tensor("out_own", [2, TO, B, P], F32, kind="ExternalOutput").ap()

    wih_sb = nc.alloc_sbuf_tensor("wih_sb", [P, 2 * KB * G], F32)
    whh_sb = nc.alloc_sbuf_tensor("whh_sb", [P, 2 * KB * G], F32)
    hbuf = nc.alloc_sbuf_tensor("hbuf", [2 * B, 2 * P], F32)
    ident_sb = nc.alloc_sbuf_tensor("ident_sb", [2 * B, 2 * B], F32)
    if with_gbias:
        gbias_sb = nc.alloc_sbuf_tensor("gbias_sb", [P, 2 * G], F32)
    if with_nbias:
        nbias_sb = nc.alloc_sbuf_tensor("nbias_sb", [2 * B, P], F32)

    with tc.tile_pool(name="dram", bufs=1, space="DRAM") as dpool:
        gid = [dpool.tile([T * B, G], F32, name=f"gid{d}") for d in (0, 1)]

        # ================= prologue =================
        for d in (0, 1):
            for k in range(KB):
                off = (d * KB + k) * G
                nc.sync.dma_start(wih_sb.ap()[:, off : off + G], wih[d, k])
                nc.sync.dma_start(whh_sb.ap()[:, off : off + G], whh[d, k])
        nc.sync.dma_start(ident_sb.ap(), ident)
        if with_gbias:
            for d in (0, 1):
                nc.sync.dma_start(gbias_sb.ap()[:, d * G : (d + 1) * G], gbias[d])
        if with_nbias:
            nc.sync.dma_start(nbias_sb.ap(), nbias)
        nc.vector.memset(hbuf.ap(), 0.0)

        # Bulk input projections: gi[d] = X2d @ Wih_d.T (own 384 cols), all t.
        with (
            tc.tile_pool(name="xtp", bufs=3) as xtp,
            tc.tile_pool(name="gps", bufs=4, space="PSUM") as gps,
            tc.tile_pool(name="gis", bufs=4) as gis,
        ):
            n_m = (T * B) // P
            for m in range(n_m):
                xtile = xtp.tile([P, KB * P], F32)
                for k in range(KB):
                    nc.sync.dma_start(
                        xtile[:, P * k : P * (k + 1)],
                        xt[P * k : P * (k + 1), m * P : (m + 1) * P],
                    )
                for d in (0, 1):
                    ps = gps.tile([P, G], F32)
                    for k in range(KB):
                        nc.tensor.matmul(
                            ps[:],
                            xtile[:, P * k : P * (k + 1)],
                            wih_sb.ap()[:, (d * KB + k) * G : (d * KB + k + 1) * G],
                            start=(k == 0),
                            stop=(k == KB - 1),
                        )
                    gt = gis.tile([P, G], F32)
                    if with_gbias:
                        nc.vector.tensor_add(
                            gt[:], ps[:], gbias_sb.ap()[:, d * G : (d + 1) * G]
                        )
                    else:
                        nc.scalar.copy(gt[:], ps[:])
                    nc.sync.dma_start(gid[d][m * P : (m + 1) * P, :], gt[:])

        # ================= scan =================
        with (
            tc.tile_pool(name="gip", bufs=6) as gip,
            tc.tile_pool(name="xop", bufs=6) as xop,
            tc.tile_pool(name="srz", bufs=3) as srzp,
            tc.tile_pool(name="rzp", bufs=3) as rzp,
            tc.tile_pool(name="sml", bufs=3) as sml,
            tc.tile_pool(name="snd", bufs=3) as sndp,
            tc.tile_pool(name="gth", bufs=3) as gthp,
            tc.tile_pool(name="cin", bufs=3, space="DRAM") as cinp,
            tc.tile_pool(name="cout", bufs=3, space="DRAM") as coutp,
            tc.tile_pool(name="pmm", bufs=3, space="PSUM") as pmm,
            tc.tile_pool(name="ptr", bufs=2, space="PSUM") as ptr,
        ):
            gth_prev = None
            for t in range(TS):
                gi_t = gip.tile([2 * B, G], F32)
                xo_t = xop.tile([2 * B, P], F32)
                for d in (0, 1):
                    idx = t if d == 0 else T - 1 - t
                    nc.sync.dma_start(
                        gi_t[d * B : (d + 1) * B, :],
                        gid[d][idx * B : (idx + 1) * B, :],
                    )
                    nc.sync.dma_start(xo_t[d * B : (d + 1) * B, :], xo[idx])

                par = t & 1
                if t == 0:
                    # h(-1) = 0 -> gh = 0: h = (1-z)*n + x
                    zc = sml.tile([2 * B, P], F32, tag="zc")
                    nc.scalar.activation(
                        zc[:],
                        gi_t[:, P : 2 * P],
                        mybir.ActivationFunctionType.Sigmoid,
                        scale=-1.0,
                    )
                    n = sml.tile([2 * B, P], F32, tag="n")
                    nc.scalar.activation(
                        n[:],
                        gi_t[:, 2 * P : 3 * P],
                        mybir.ActivationFunctionType.Tanh,
                    )
                    u1 = sml.tile([2 * B, P], F32, tag="u1")
                    nc.vector.tensor_mul(u1[:], zc[:], n[:])
                    hn = hbuf.ap()[:, par * P : (par + 1) * P]
                    nc.vector.tensor_add(hn, u1[:], xo_t[:])
                else:
                    pp = (t - 1) & 1
                    ps = pmm.tile([2 * B, G], F32)
                    for d in (0, 1):
                        for k in range(KB):
                            nc.tensor.matmul(
                                ps[d * B : (d + 1) * B, :],
                                gth_prev[:, (d * NC + k) * B : (d * NC + k + 1) * B],
                                whh_sb.ap()[
                                    :, (d * KB + k) * G : (d * KB + k + 1) * G
                                ],
                                start=(k == 0),
                                stop=(k == KB - 1),
                                tile_position=(0, d * B),
                                skip_group_check=True,
                            )
                    s_rz = srzp.tile([2 * B, 2 * P], F32)
                    nc.vector.tensor_add(s_rz[:], gi_t[:, : 2 * P], ps[:, : 2 * P])
                    rz = rzp.tile([2 * B, 2 * P], F32)
                    nc.scalar.activation(
                        rz[:], s_rz[:], mybir.ActivationFunctionType.Sigmoid
                    )
                    zc = sml.tile([2 * B, P], F32, tag="zc")
                    nc.scalar.activation(
                        zc[:],
                        s_rz[:, P : 2 * P],
                        mybir.ActivationFunctionType.Sigmoid,
                        scale=-1.0,
                    )
                    gn = ps[:, 2 * P : 3 * P]
                    if with_nbias:
                        gnb = sml.tile([2 * B, P], F32, tag="gnb")
                        nc.vector.tensor_add(gnb[:], gn, nbias_sb.ap())
                        gn = gnb[:]
                    t1 = sml.tile([2 * B, P], F32, tag="t1")
                    nc.vector.tensor_mul(t1[:], rz[:, :P], gn)
                    t2 = sml.tile([2 * B, P], F32, tag="t2")
                    nc.vector.tensor_add(t2[:], t1[:], gi_t[:, 2 * P : 3 * P])
                    n = sml.tile([2 * B, P], F32, tag="n")
                    nc.scalar.activation(
                        n[:], t2[:], mybir.ActivationFunctionType.Tanh
                    )
                    zh = sml.tile([2 * B, P], F32, tag="zh")
                    nc.vector.tensor_mul(
                        zh[:], rz[:, P : 2 * P], hbuf.ap()[:, pp * P : (pp + 1) * P]
                    )
                    u1 = sml.tile([2 * B, P], F32, tag="u1")
                    nc.vector.tensor_mul(u1[:], zc[:], n[:])
                    u2 = sml.tile([2 * B, P], F32, tag="u2")
                    nc.vector.tensor_add(u2[:], u1[:], zh[:])
                    hn = hbuf.ap()[:, par * P : (par + 1) * P]
                    nc.vector.tensor_add(hn, u2[:], xo_t[:])

                if L <= t < T - L:
                    for d in (0, 1):
                        nc.sync.dma_start(outp[d, t - L], hn[d * B : (d + 1) * B, :])

                # --- exchange h.T chunks via AllGather (skip on final step) ---
                if t == TS - 1:
                    continue
                tp = ptr.tile([P, 2 * B], F32)
                nc.tensor.transpose(tp[:], hn, ident_sb.ap())
                snd = sndp.tile([P, 2 * B], F32)
                nc.scalar.copy(snd[:], tp[:])
                cin = cinp.tile([P, 2 * B], F32)
                nc.sync.dma_start(cin[:], snd[:])
                cout = coutp.tile([NC * P, 2 * B], F32, addr_space="Shared")
                nc.gpsimd.collective_compute(
                    "AllGather",
                    mybir.AluOpType.bypass,
                    replica_groups=[list(range(NC))],
                    ins=[cin.opt()],
                    outs=[cout.opt()],
                )
                # gathered h.T back to SBUF: [128, (d, k, B)] with slot k from
                # rank k's rows [128k:128k+128], cols d*B:(d+1)*B
                gth = gthp.tile([P, 2 * NC * B], F32)
                for d in (0, 1):
                    for k in range(NC):
                        nc.sync.dma_start(
                            gth[:, (d * NC + k) * B : (d * NC + k + 1) * B],
                            cout[k * P : (k + 1) * P, d * B : (d + 1) * B],
                        )
                gth_prev = gth
    return []


def patch_deferred_waits(nc, deferred):
    assert not deferred


def make_in_maps(inputs: dict, core: int) -> dict:
    x = np.asarray(inputs["input_x"], np.float32)[:, :, :F]  # [B, T, F]
    own = slice(core * P, (core + 1) * P)

    def own_cols(w):  # [3F, F] -> W.T own cols [F, 384]
        wt = np.ascontiguousarray(np.asarray(w, np.float32).T)
        return np.concatenate(
            [wt[:, g * F + core * P : g * F + (core + 1) * P] for g in range(3)],
            axis=1,
        )

    def own_vec(v):
        v = np.asarray(v, np.float32)
        return np.concatenate(
            [v[g * F + core * P : g * F + (core + 1) * P] for g in range(3)]
        )

    m = {
        "xt": np.ascontiguousarray(x.transpose(2, 1, 0).reshape(F, T * B)),
        "xo": np.ascontiguousarray(x.transpose(1, 0, 2)[:, :, own]),
        "wih": np.ascontiguousarray(
            np.stack(
                [own_cols(inputs["Wih_f"]).reshape(KB, P, G),
                 own_cols(inputs["Wih_b"]).reshape(KB, P, G)]
            )
        ),
        "whh": np.ascontiguousarray(
            np.stack(
                [own_cols(inputs["Whh_f"]).reshape(KB, P, G),
                 own_cols(inputs["Whh_b"]).reshape(KB, P, G)]
            )
        ),
        "ident": np.eye(2 * B, dtype=np.float32),
    }
    # gate biases: bih (all gates) + bhh (r,z only) fold into gi; bhh_n is
    # applied inside the n-gate (it is multiplied by r together with gh_n).
    gb = []
    nb = []
    for d, (bi, bh) in enumerate(
        [(inputs["bih_f"], inputs["bhh_f"]), (inputs["bih_b"], inputs["bhh_b"])]
    ):
        bio, bho = own_vec(bi), own_vec(bh)
        gv = bio.copy()
        gv[: 2 * P] += bho[: 2 * P]
        gb.append(np.broadcast_to(gv, (P, G)))
        nb.append(np.broadcast_to(bho[2 * P :], (B, P)))
    m["_gbias"] = np.ascontiguousarray(np.stack(gb))  # [2, P, G]
    m["_nbias"] = np.ascontiguousarray(np.concatenate(nb, axis=0))  # [2B, P]
    return m


_COMPILED = {}


def _get_compiled(with_gbias: bool, with_nbias: bool):
    key = (with_gbias, with_nbias)
    if key not in _COMPILED:
        nc = bacc.Bacc(
            "TRN2",
            target_bir_lowering=False,
            debug=False,
            enable_asserts=True,
            num_devices=NC,
        )
        with tile.TileContext(nc) as tc:
            deferred = build_gru_kernel(nc, tc, with_gbias, with_nbias)
        patch_deferred_waits(nc, deferred)
        nc.compile()
        _COMPILED[key] = nc
    return _COMPILED[key]


def kernel(**inputs) -> np.ndarray:
    maps = [make_in_maps(inputs, c) for c in range(NC)]
    with_gbias = any(np.any(m["_gbias"]) for m in maps)
    with_nbias = any(np.any(m["_nbias"]) for m in maps)
    in_maps = []
    for m in maps:
        gb, nb = m.pop("_gbias"), m.pop("_nbias")
        if with_gbias:
            m["gbias"] = gb
        if with_nbias:
            m["nbias"] = nb
        in_maps.append(m)

    nc = _get_compiled(with_gbias, with_nbias)
    res = bass_utils.run_bass_kernel_spmd(nc, in_maps, core_ids=list(range(NC)))

    TO = T - 2 * L
    out = np.empty((B, TO, 2 * F), np.float32)
    for c in range(NC):
        oo = np.asarray(res.results[c]["out_own"])  # [2, TO, B, P]
        out[:, :, c * P : (c + 1) * P] = oo[0].transpose(1, 0, 2)
        out[:, :, F + c * P : F + (c + 1) * P] = oo[1].transpose(1, 0, 2)
    return out


# revision 3
# speedup vs baseline: 1.2190x; 1.2190x over previous
"""BiGRU encoder (nn_BiGRUEncoder) as an 8-core TRN2 Bass kernel.

Contract: kernel(**inputs) takes the FULL unsharded inputs from
setup_inputs() and returns the FULL [B, T-2L, 2F] output, distributing work
across 8 NeuronCores internally.

Decomposition: the hidden dim F=1024 is split across the 8 cores (128
features each). Every core runs BOTH scan directions with the full batch
B=32, computing its 384 rows of the 3F gate pre-activations per step. After
each step the transposed h chunks ([128, 32] per direction) are exchanged
with an AllGather so the next step's recurrent matmul has the full h.T.
Input projections gi = x @ Wih.T don't depend on h and are hoisted into a
prologue as one large batched matmul per direction, stored in DRAM, and
streamed per step.

Per-step layouts: batch on partitions for gate math, with both directions
stacked ([64, X]: fwd rows 0-31, bwd rows 32-63); features on partitions for
the exchanged h.T chunks. The scan stops at T-L: the last L steps of either
direction feed no output.
"""

import sys

sys.path.insert(0, "/opt/trn_rl_repo")

import os

import numpy as np

from concourse import bacc, tile, mybir
from concourse import bass_utils

F32 = mybir.dt.float32

B = 32  # batch
T = 512  # sequence length
F = 1024  # hidden/feature dim
L = 10  # trim at both ends of T
NC = 8  # cores
P = 128  # partitions / features per core
G = 3 * P  # gate rows per core
KB = F // P  # contraction blocks


def build_gru_kernel(nc, tc, with_gbias: bool, with_nbias: bool):
    """Emit the SPMD program (identical on all 8 cores)."""
    ablate = os.environ.get("K_ABLATE", "")
    TS = 1 if ablate == "prologue" else T - L  # scan steps needed
    TO = T - 2 * L  # output steps

    xt = nc.dram_tensor("xt", [F, T * B], F32, kind="ExternalInput").ap()
    xo = nc.dram_tensor("xo", [T, B, P], F32, kind="ExternalInput").ap()
    wih = nc.dram_tensor("wih", [2, KB, P, G], F32, kind="ExternalInput").ap()
    whh = nc.dram_tensor("whh", [2, KB, P, G], F32, kind="ExternalInput").ap()
    ident = nc.dram_tensor("ident", [2 * B, 2 * B], F32, kind="ExternalInput").ap()
    if with_gbias:
        gbias = nc.dram_tensor("gbias", [2, P, G], F32, kind="ExternalInput").ap()
    if with_nbias:
        nbias = nc.dram_tensor("nbias", [2 * B, P], F32, kind="ExternalInput").ap()
    outp = nc.dram_tensor("out_own", [2, TO, B, P], F32, kind="ExternalOutput").ap()

    wih_sb = nc.alloc_sbuf_tensor("wih_sb", [P, 2 * KB * G], F32)
    whh_sb = nc.alloc_sbuf_tensor("whh_sb", [P, 2 * KB * G], F32)
    hbuf = nc.alloc_sbuf_tensor("hbuf", [2 * B, 2 * P], F32)
    ident_sb = nc.alloc_sbuf_tensor("ident_sb", [2 * B, 2 * B], F32)
    if with_gbias:
        gbias_sb = nc.alloc_sbuf_tensor("gbias_sb", [P, 2 * G], F32)
    if with_nbias:
        nbias_sb = nc.alloc_sbuf_tensor("nbias_sb", [2 * B, P], F32)

    with tc.tile_pool(name="dram", bufs=1, space="DRAM") as dpool:
        gid = [dpool.tile([T * B, G], F32, name=f"gid{d}") for d in (0, 1)]

        # ================= prologue =================
        for d in (0, 1):
            for k in range(KB):
                off = (d * KB + k) * G
                nc.sync.dma_start(wih_sb.ap()[:, off : off + G], wih[d, k])
                nc.sync.dma_start(whh_sb.ap()[:, off : off + G], whh[d, k])
        nc.sync.dma_start(ident_sb.ap(), ident)
        if with_gbias:
            for d in (0, 1):
                nc.sync.dma_start(gbias_sb.ap()[:, d * G : (d + 1) * G], gbias[d])
        if with_nbias:
            nc.sync.dma_start(nbias_sb.ap(), nbias)
        nc.vector.memset(hbuf.ap(), 0.0)

        # Bulk input projections: gi[d] = X2d @ Wih_d.T (own 384 cols), all t.
        with (
            tc.tile_pool(name="xtp", bufs=3) as xtp,
            tc.tile_pool(name="gps", bufs=4, space="PSUM") as gps,
            tc.tile_pool(name="gis", bufs=4) as gis,
        ):
            n_m = (T * B) // P
            for m in range(n_m):
                xtile = xtp.tile([P, KB * P], F32)
                for k in range(KB):
                    nc.sync.dma_start(
                        xtile[:, P * k : P * (k + 1)],
                        xt[P * k : P * (k + 1), m * P : (m + 1) * P],
                    )
                for d in (0, 1):
                    ps = gps.tile([P, G], F32)
                    for k in range(KB):
                        nc.tensor.matmul(
                            ps[:],
                            xtile[:, P * k : P * (k + 1)],
                            wih_sb.ap()[:, (d * KB + k) * G : (d * KB + k + 1) * G],
                            start=(k == 0),
                            stop=(k == KB - 1),
                        )
                    gt = gis.tile([P, G], F32)
                    if with_gbias:
                        nc.vector.tensor_add(
                            gt[:], ps[:], gbias_sb.ap()[:, d * G : (d + 1) * G]
                        )
                    else:
                        nc.scalar.copy(gt[:], ps[:])
                    nc.sync.dma_start(gid[d][m * P : (m + 1) * P, :], gt[:])

        # ================= scan =================
        with (
            tc.tile_pool(name="gip", bufs=6) as gip,
            tc.tile_pool(name="xop", bufs=6) as xop,
            tc.tile_pool(name="srz", bufs=3) as srzp,
            tc.tile_pool(name="rzp", bufs=3) as rzp,
            tc.tile_pool(name="sml", bufs=3) as sml,
            tc.tile_pool(name="snd", bufs=3) as sndp,
            tc.tile_pool(name="gth", bufs=3) as gthp,
            tc.tile_pool(name="cin", bufs=3, space="DRAM") as cinp,
            tc.tile_pool(name="cout", bufs=3, space="DRAM") as coutp,
            tc.tile_pool(name="pmm", bufs=3, space="PSUM") as pmm,
            tc.tile_pool(name="ptr", bufs=2, space="PSUM") as ptr,
        ):
            gth_prev = None
            for t in range(TS):
                gi_t = gip.tile([2 * B, G], F32)
                xo_t = xop.tile([2 * B, P], F32)
                for d in (0, 1):
                    idx = t if d == 0 else T - 1 - t
                    nc.sync.dma_start(
                        gi_t[d * B : (d + 1) * B, :],
                        gid[d][idx * B : (idx + 1) * B, :],
                    )
                    nc.sync.dma_start(xo_t[d * B : (d + 1) * B, :], xo[idx])

                par = t & 1
                if t == 0:
                    # h(-1) = 0 -> gh = 0: h = (1-z)*n + x
                    zc = sml.tile([2 * B, P], F32, tag="zc")
                    nc.scalar.activation(
                        zc[:],
                        gi_t[:, P : 2 * P],
                        mybir.ActivationFunctionType.Sigmoid,
                        scale=-1.0,
                    )
                    n = sml.tile([2 * B, P], F32, tag="n")
                    nc.scalar.activation(
                        n[:],
                        gi_t[:, 2 * P : 3 * P],
                        mybir.ActivationFunctionType.Tanh,
                    )
                    u1 = sml.tile([2 * B, P], F32, tag="u1")
                    nc.vector.tensor_mul(u1[:], zc[:], n[:])
                    hn = hbuf.ap()[:, par * P : (par + 1) * P]
                    nc.vector.tensor_add(hn, u1[:], xo_t[:])
                else:
                    pp = (t - 1) & 1
                    ps = pmm.tile([2 * B, G], F32)
                    for d in (0, 1):
                        for k in range(KB):
                            nc.tensor.matmul(
                                ps[d * B : (d + 1) * B, :],
                                gth_prev[:, (d * NC + k) * B : (d * NC + k + 1) * B],
                                whh_sb.ap()[
                                    :, (d * KB + k) * G : (d * KB + k + 1) * G
                                ],
                                start=(k == 0),
                                stop=(k == KB - 1),
                                tile_position=(0, d * B),
                                skip_group_check=True,
                            )
                    s_rz = srzp.tile([2 * B, 2 * P], F32)
                    nc.vector.tensor_add(s_rz[:], gi_t[:, : 2 * P], ps[:, : 2 * P])
                    rz = rzp.tile([2 * B, 2 * P], F32)
                    nc.scalar.activation(
                        rz[:], s_rz[:], mybir.ActivationFunctionType.Sigmoid
                    )
                    zc = sml.tile([2 * B, P], F32, tag="zc")
                    nc.scalar.activation(
                        zc[:],
                        s_rz[:, P : 2 * P],
                        mybir.ActivationFunctionType.Sigmoid,
                        scale=-1.0,
                    )
                    gn = ps[:, 2 * P : 3 * P]
                    if with_nbias:
                        gnb = sml.tile([2 * B, P], F32, tag="gnb")
                        nc.vector.tensor_add(gnb[:], gn, nbias_sb.ap())
                        gn = gnb[:]
                    t1 = sml.tile([2 * B, P], F32, tag="t1")
                    nc.vector.tensor_mul(t1[:], rz[:, :P], gn)
                    t2 = sml.tile([2 * B, P], F32, tag="t2")
                    nc.vector.tensor_add(t2[:], t1[:], gi_t[:, 2 * P : 3 * P])
                    n = sml.tile([2 * B, P], F32, tag="n")
                    nc.scalar.activation(
                        n[:], t2[:], mybir.ActivationFunctionType.Tanh
                    )
                    zh = sml.tile([2 * B, P], F32, tag="zh")
                    nc.vector.tensor_mul(
                        zh[:], rz[:, P : 2 * P], hbuf.ap()[:, pp * P : (pp + 1) * P]
                    )
                    u1 = sml.tile([2 * B, P], F32, tag="u1")
                    nc.vector.tensor_mul(u1[:], zc[:], n[:])
                    u2 = sml.tile([2 * B, P], F32, tag="u2")
                    nc.vector.tensor_add(u2[:], u1[:], zh[:])
                    hn = hbuf.ap()[:, par * P : (par + 1) * P]
                    nc.vector.tensor_add(hn, u2[:], xo_t[:])

                if L <= t < T - L:
                    for d in (0, 1):
                        nc.sync.dma_start(outp[d, t - L], hn[d * B : (d + 1) * B, :])

                # --- exchange h.T chunks via AllGather (skip on final step) ---
                if t == TS - 1:
                    continue
                tp = ptr.tile([P, 2 * B], F32)
                nc.tensor.transpose(tp[:], hn, ident_sb.ap())
                snd = sndp.tile([P, 2 * B], F32)
                nc.scalar.copy(snd[:], tp[:])
                if ablate == "noexch":
                    if gth_prev is None:
                        gth = gthp.tile([P, 2 * NC * B], F32)
                        for k in range(2 * NC):
                            nc.vector.tensor_copy(
                                gth[:, k * B : (k + 1) * B], snd[:, :B]
                            )
                        gth_prev = gth
                    continue
                cin = cinp.tile([P, 2 * B], F32)
                nc.sync.dma_start(cin[:], snd[:])
                cout = coutp.tile([NC * P, 2 * B], F32, addr_space="Shared")
                nc.gpsimd.collective_compute(
                    "AllGather",
                    mybir.AluOpType.bypass,
                    replica_groups=[list(range(NC))],
                    ins=[cin.opt()],
                    outs=[cout.opt()],
                )
                # gathered h.T back to SBUF: [128, (d, k, B)] with slot k from
                # rank k's rows [128k:128k+128], cols d*B:(d+1)*B
                gth = gthp.tile([P, 2 * NC * B], F32)
                for d in (0, 1):
                    for k in range(NC):
                        nc.sync.dma_start(
                            gth[:, (d * NC + k) * B : (d * NC + k + 1) * B],
                            cout[k * P : (k + 1) * P, d * B : (d + 1) * B],
                        )
                gth_prev = gth
    return []


def patch_deferred_waits(nc, deferred):
    assert not deferred


def make_in_maps(inputs: dict, core: int) -> dict:
    x = np.asarray(inputs["input_x"], np.float32)[:, :, :F]  # [B, T, F]
    own = slice(core * P, (core + 1) * P)

    def own_cols(w):  # [3F, F] -> W.T own cols [F, 384]
        wt = np.ascontiguousarray(np.asarray(w, np.float32).T)
        return np.concatenate(
            [wt[:, g * F + core * P : g * F + (core + 1) * P] for g in range(3)],
            axis=1,
        )

    def own_vec(v):
        v = np.asarray(v, np.float32)
        return np.concatenate(
            [v[g * F + core * P : g * F + (core + 1) * P] for g in range(3)]
        )

    m = {
        "xt": np.ascontiguousarray(x.transpose(2, 1, 0).reshape(F, T * B)),
        "xo": np.ascontiguousarray(x.transpose(1, 0, 2)[:, :, own]),
        "wih": np.ascontiguousarray(
            np.stack(
                [own_cols(inputs["Wih_f"]).reshape(KB, P, G),
                 own_cols(inputs["Wih_b"]).reshape(KB, P, G)]
            )
        ),
        "whh": np.ascontiguousarray(
            np.stack(
                [own_cols(inputs["Whh_f"]).reshape(KB, P, G),
                 own_cols(inputs["Whh_b"]).reshape(KB, P, G)]
            )
        ),
        "ident": np.eye(2 * B, dtype=np.float32),
    }
    # gate biases: bih (all gates) + bhh (r,z only) fold into gi; bhh_n is
    # applied inside the n-gate (it is multiplied by r together with gh_n).
    gb = []
    nb = []
    for d, (bi, bh) in enumerate(
        [(inputs["bih_f"], inputs["bhh_f"]), (inputs["bih_b"], inputs["bhh_b"])]
    ):
        bio, bho = own_vec(bi), own_vec(bh)
        gv = bio.copy()
        gv[: 2 * P] += bho[: 2 * P]
        gb.append(np.broadcast_to(gv, (P, G)))
        nb.append(np.broadcast_to(bho[2 * P :], (B, P)))
    m["_gbias"] = np.ascontiguousarray(np.stack(gb))  # [2, P, G]
    m["_nbias"] = np.ascontiguousarray(np.concatenate(nb, axis=0))  # [2B, P]
    return m


_COMPILED = {}


def _get_compiled(with_gbias: bool, with_nbias: bool):
    key = (with_gbias, with_nbias, os.environ.get("K_ABLATE", ""))
    if key not in _COMPILED:
        nc = bacc.Bacc(
            "TRN2",
            target_bir_lowering=False,
            debug=False,
            enable_asserts=True,
            num_devices=NC,
        )
        with tile.TileContext(nc) as tc:
            deferred = build_gru_kernel(nc, tc, with_gbias, with_nbias)
        patch_deferred_waits(nc, deferred)
        nc.compile()
        _COMPILED[key] = nc
    return _COMPILED[key]


def kernel(**inputs) -> np.ndarray:
    maps = [make_in_maps(inputs, c) for c in range(NC)]
    with_gbias = any(np.any(m["_gbias"]) for m in maps)
    with_nbias = any(np.any(m["_nbias"]) for m in maps)
    in_maps = []
    for m in maps:
        gb, nb = m.pop("_gbias"), m.pop("_nbias")
        if with_gbias:
            m["gbias"] = gb
        if with_nbias:
            m["nbias"] = nb
        in_maps.append(m)

    nc = _get_compiled(with_gbias, with_nbias)
    res = bass_utils.run_bass_kernel_spmd(nc, in_maps, core_ids=list(range(NC)))

    TO = T - 2 * L
    out = np.empty((B, TO, 2 * F), np.float32)
    for c in range(NC):
        oo = np.asarray(res.results[c]["out_own"])  # [2, TO, B, P]
        out[:, :, c * P : (c + 1) * P] = oo[0].transpose(1, 0, 2)
        out[:, :, F + c * P : F + (c + 1) * P] = oo[1].transpose(1, 0, 2)
    return out
